# revision 1
# baseline (speedup 1.0000x reference)
"""CBOW forward (embedding lookup -> ReLU -> vocab projection) on 8 TRN2 cores.

Full inputs in, full output out.  Sharding: every core computes the full
hidden layer h redundantly (the gather is tiny next to the output), and
the vocab axis of layer 2 is sharded 8 ways: core c computes
out[:, c*6250:(c+1)*6250] = h @ W2_c.T + b2_c.

Layer 1 (embedding sum): 8 independent indirect row-gathers per 128-row
block (one W1T row per partition per call — the only layout HW SWDGE
honors) land in an [n, 8, d] SBUF tile; a 3-level DVE tree sums them.
Duplicate context indices use scatter-SET semantics (count once): the
host redirects duplicate occurrences to an appended all-zero row of
W1T.  Two PE transposes per block produce hT, with ReLU(x/8 + b1) fused
into the PSUM eviction on the Scalar engine.

Layer 2 uses the 3-term bf16 split (h_hi@W_hi + h_hi@W_lo + h_lo@W_hi,
fp32 PSUM accumulate): fp32-grade accuracy (~6e-6 scale-relative) at
bf16 TensorEngine rates.  b2 is broadcast to all partitions once via a
partition-step-0 DMA and added during the fp32 PSUM eviction on the
DVE.  Emission is software-pipelined (layer1 of block nb+1 ahead of
layer2 of block nb) to keep the three engine FIFOs from head-of-line
blocking each other.
"""

from contextlib import ExitStack

import numpy as np
import ml_dtypes

import concourse.bacc as bacc
import concourse.bass as bass
import concourse.mybir as mybir
import concourse.tile as tile
from concourse.masks import make_identity

# Problem shape (hardcoded per the task contract).
N = 2048          # batch
J = 8             # context window (2*CTX)
D = 256           # hidden
V = 50000         # vocab
C = 8             # cores
VS = V // C       # vocab shard per core = 6250

P = 128
NB = N // P       # 16 row-blocks
VT = 1024         # output tile width (two PSUM banks)
SPLIT_LO = True   # 3-term bf16 split (False: plain bf16, 1 term per half)

F32 = mybir.dt.float32
BF16 = mybir.dt.bfloat16
I32 = mybir.dt.int32

_CACHE = {}


def _build():
    """Build + compile the single-core SPMD Bass program."""
    if "nc" in _CACHE:
        return _CACHE["nc"]

    nc = bacc.Bacc("TRN2", target_bir_lowering=False, debug=False, num_devices=C)

    idx_d = nc.dram_tensor("idx", [P, NB * J], I32, kind="ExternalInput")
    w1t_d = nc.dram_tensor("w1t", [V + 1, D], F32, kind="ExternalInput")
    w2thi_d = nc.dram_tensor("w2thi", [D, VS], BF16, kind="ExternalInput")
    w2tlo_d = nc.dram_tensor("w2tlo", [D, VS], BF16, kind="ExternalInput")
    b1_d = nc.dram_tensor("b1", [2, P, 1], F32, kind="ExternalInput")
    b2_d = nc.dram_tensor("b2", [1, VS], F32, kind="ExternalInput")
    out_d = nc.dram_tensor("out", [N, VS], F32, kind="ExternalOutput")

    vsub = [(v, min(VT, VS - v)) for v in range(0, VS, VT)]  # 6x1024 + 106

    with tile.TileContext(nc) as tc, ExitStack() as ctx:
        const = ctx.enter_context(tc.tile_pool(name="const", bufs=1))
        gpool = ctx.enter_context(tc.tile_pool(name="g8", bufs=3))
        t4pool = ctx.enter_context(tc.tile_pool(name="t4", bufs=2))
        t2pool = ctx.enter_context(tc.tile_pool(name="t2", bufs=2))
        hpool = ctx.enter_context(tc.tile_pool(name="hraw", bufs=2))
        hfpool = ctx.enter_context(tc.tile_pool(name="hf", bufs=2))
        htpool = ctx.enter_context(tc.tile_pool(name="ht", bufs=4))
        opool = ctx.enter_context(tc.tile_pool(name="out", bufs=4))
        ps_s = ctx.enter_context(tc.tile_pool(name="ps_s", bufs=2, space="PSUM"))
        ps_b = ctx.enter_context(tc.tile_pool(name="ps_b", bufs=3, space="PSUM"))

        # ---- resident tensors -------------------------------------------
        idx_sb = const.tile([P, NB * J], I32, tag="idx")
        # block-0 indices first so its gathers start ~6us earlier
        nc.sync.dma_start(idx_sb[:, :J], idx_d[:, :J])
        nc.sync.dma_start(idx_sb[:, J:], idx_d[:, J:])
        ident = const.tile([P, P], F32, tag="ident")
        make_identity(nc, ident[:])
        b1t = [const.tile([P, 1], F32, tag=f"b1{h}", name=f"b1{h}")
               for h in (0, 1)]
        for h in (0, 1):
            nc.sync.dma_start(b1t[h][:], b1_d[h])
        w2hi = [const.tile([P, VS], BF16, tag=f"w2hi{h}", name=f"w2hi{h}")
                for h in (0, 1)]
        w2lo = [const.tile([P, VS], BF16, tag=f"w2lo{h}", name=f"w2lo{h}")
                for h in (0, 1)]
        for h in (0, 1):
            nc.sync.dma_start(w2hi[h][:], w2thi_d[h * P:(h + 1) * P, :])
            if SPLIT_LO:
                nc.sync.dma_start(w2lo[h][:], w2tlo_d[h * P:(h + 1) * P, :])
        # b2 broadcast to all 128 partitions via partition-step-0 DMA
        b2bc = const.tile([P, VS], F32, tag="b2bc")
        nc.sync.dma_start(b2bc[:], b2_d[:].to_broadcast([P, VS]))

        # ---- main pipeline ----------------------------------------------
        def layer1(nb):
            # 8 independent slice-gathers (chained CCE-accumulate gathers
            # serialize the SWDGE queue at 2us/call; independent calls run
            # at ~1.1us), then a 3-level DVE tree sum.
            g8 = gpool.tile([P, J, D], F32, tag="g8", name="g8")
            for j in range(J):
                nc.gpsimd.indirect_dma_start(
                    out=g8[:, j, :],
                    out_offset=None,
                    in_=w1t_d[:],
                    in_offset=bass.IndirectOffsetOnAxis(
                        ap=idx_sb[:, nb * J + j:nb * J + j + 1], axis=0),
                )
            t4 = t4pool.tile([P, 4, D], F32, tag="t4", name="t4")
            nc.vector.tensor_add(t4[:], g8[:, 0:4, :], g8[:, 4:8, :])
            t2 = t2pool.tile([P, 2, D], F32, tag="t2", name="t2")
            nc.vector.tensor_add(t2[:], t4[:, 0:2, :], t4[:, 2:4, :])
            h_raw = hpool.tile([P, D], F32, tag="hraw", name="h_raw")
            nc.vector.tensor_add(h_raw[:], t2[:, 0, :], t2[:, 1, :])

            # transpose to hT, fuse relu(x/8 + b1), emit bf16 hi/lo halves.
            # Separate tiles per half so h=0 matmuls never wait on h=1's
            # transpose/relu/sub chain.
            ht_hi = [htpool.tile([P, P], BF16, tag=f"hthi{h}", name=f"ht_hi{h}")
                     for h in (0, 1)]
            ht_lo = [htpool.tile([P, P], BF16, tag=f"htlo{h}", name=f"ht_lo{h}")
                     for h in (0, 1)]
            for h in (0, 1):
                pt = ps_s.tile([P, 512], F32, tag="ps", name="pt")
                nc.tensor.transpose(pt[:, :P], h_raw[:, h * P:(h + 1) * P],
                                    ident[:])
                nc.scalar.activation(ht_hi[h][:], pt[:, :P],
                                     mybir.ActivationFunctionType.Relu,
                                     bias=b1t[h][:], scale=1.0 / J)
                if SPLIT_LO:
                    hf = hfpool.tile([P, P], F32, tag="hf", name="hf")
                    nc.scalar.activation(hf[:], pt[:, :P],
                                         mybir.ActivationFunctionType.Relu,
                                         bias=b1t[h][:], scale=1.0 / J)
                    nc.vector.tensor_tensor(
                        out=ht_lo[h][:], in0=hf[:], in1=ht_hi[h][:],
                        op=mybir.AluOpType.subtract)
            return ht_hi, ht_lo

        def layer2(nb, ht_hi, ht_lo):
            terms = ([(ht_hi, w2hi), (ht_hi, w2lo), (ht_lo, w2hi)]
                     if SPLIT_LO else [(ht_hi, w2hi)])
            for v0, vw in vsub:
                po = ps_b.tile([P, VT], F32, tag="po", name="po")
                for sub in range(0, vw, 512):
                    sw = min(512, vw - sub)
                    for h in (0, 1):
                        for t, (hta, w2a) in enumerate(terms):
                            nc.tensor.matmul(
                                po[:, sub:sub + sw],
                                lhsT=hta[h][:],
                                rhs=w2a[h][:, v0 + sub:v0 + sub + sw],
                                start=(h == 0 and t == 0),
                                stop=(h == 1 and t == len(terms) - 1))
                ot = opool.tile([P, VT], F32, tag="ot", name="ot")
                nc.vector.tensor_add(ot[:, :vw], po[:, :vw], b2bc[:, v0:v0 + vw])
                nc.sync.dma_start(out_d[nb * P:(nb + 1) * P, v0:v0 + vw],
                                  ot[:, :vw])

        # software-pipelined emission: layer1(nb+1) ahead of layer2(nb) so
        # the next block's DVE tree isn't stuck behind this block's
        # eviction burst in the DVE FIFO.  Depth 1 measured best: deeper
        # prefetch re-creates DVE priority inversion (layer-1 ops of far
        # blocks delay evictions, stalling PE on PSUM slot recycling).
        DEPTH = 1
        hts = {nb: layer1(nb) for nb in range(min(DEPTH, NB))}
        for nb in range(NB):
            if nb + DEPTH < NB:
                hts[nb + DEPTH] = layer1(nb + DEPTH)
            layer2(nb, *hts.pop(nb))

    nc.compile()
    _CACHE["nc"] = nc
    return nc


def _host_prep(inputs, W1, b1, W2, b2):
    x = np.asarray(inputs)
    assert x.shape == (N, J) and x.dtype == np.int32

    # duplicate mask: scatter-SET semantics -> only first occurrence counts;
    # duplicates are redirected to the all-zero row V of the augmented W1T.
    dup = np.zeros((N, J), dtype=bool)
    for j in range(1, J):
        dup[:, j] = (x[:, :j] == x[:, j:j + 1]).any(axis=1)
    xd = np.where(dup, V, x).astype(np.int32)

    # idx2[p, nb*J + j] = xd[nb*128 + p, j]
    idx2 = np.ascontiguousarray(
        xd.reshape(NB, P, J).transpose(1, 0, 2).reshape(P, NB * J))

    w1 = np.asarray(W1, dtype=np.float32)
    w1t = np.concatenate([w1.T, np.zeros((1, D), np.float32)], axis=0)
    w1t = np.ascontiguousarray(w1t)                          # [V+1, D]

    w2t = np.ascontiguousarray(np.asarray(W2, dtype=np.float32).T)  # [D, V]
    w2t_hi = w2t.astype(ml_dtypes.bfloat16)
    w2t_lo = (w2t - w2t_hi.astype(np.float32)).astype(ml_dtypes.bfloat16)

    b1r = np.ascontiguousarray(np.asarray(b1, dtype=np.float32).reshape(2, P, 1))
    b2f = np.asarray(b2, dtype=np.float32)

    in_maps = []
    for c in range(C):
        sl = slice(c * VS, (c + 1) * VS)
        in_maps.append({
            "idx": idx2,
            "w1t": w1t,
            "w2thi": np.ascontiguousarray(w2t_hi[:, sl]),
            "w2tlo": np.ascontiguousarray(w2t_lo[:, sl]),
            "b1": b1r,
            "b2": np.ascontiguousarray(b2f[sl].reshape(1, VS)),
        })
    return in_maps


def run(inputs, W1, b1, W2, b2, trace=False):
    from concourse.bass_utils import run_bass_kernel_spmd

    nc = _build()
    in_maps = _host_prep(inputs, W1, b1, W2, b2)
    res = run_bass_kernel_spmd(nc, in_maps, core_ids=list(range(C)), trace=trace)
    out = np.concatenate([res.results[c]["out"] for c in range(C)], axis=1)
    return out, res


def kernel(inputs, W1, b1, W2, b2):
    out, _ = run(inputs, W1, b1, W2, b2, trace=False)
    return out



# revision 3
# speedup vs baseline: 1.9512x; 1.9512x over previous
"""CBOW forward (embedding lookup -> ReLU -> vocab projection) on 8 TRN2 cores.

Full inputs in, full output out.  Two-level sharding:
  - layer 1 (embedding sum) is batch-sharded: core c gathers + reduces
    only its two 128-row blocks (16 indirect-DMA calls instead of 128 --
    the HW-DGE indirect path costs ~1.4us of gpsimd issue time per call
    regardless of size, so call count is what matters), relu's the
    transposed result into bf16 hT tiles and AllGathers them (2x 128KB
    collectives, staggered per block so layer 2 can start early).
  - layer 2 (the 52 GFLOP vocab projection) is vocab-sharded: every core
    computes out[:, c*6250:(c+1)*6250] = h @ W2_c.T for all 16 blocks,
    reading hT tiles back from the gathered (Shared-DRAM) buffer.

The harness accuracy gate is rel_err < 2e-2; the whole pipeline runs in
bf16 (measured ~4e-3): W1T is pre-scaled by 1/8 (exact in bf16) and
stored bf16, the context sum is a 3-level bf16 DVE tree, layer 2 is a
single bf16 term with fp32 PSUM accumulate, and the output is written
bf16 (halves the dominant HBM traffic; host upcasts).  PSUM eviction
(fp32->bf16, ~1ns/col/partition) is split DVE/Scalar per 1024-col tile
so neither engine becomes the bottleneck.  Output leaves as one
[128, 6250] DMA per block (12.5KB/row descriptors keep DMA engines at
full rate).

Duplicate context indices use scatter-SET semantics (count once): the
host redirects duplicate occurrences to an appended all-zero row of
W1T.  b1/b2 are zero in this problem (spec fill=zeros); a general
fallback path (scalar-relu with b1 bias, all-DVE b2-add evictions)
compiles only if nonzero biases ever show up.
"""

from contextlib import ExitStack

import numpy as np
import ml_dtypes

import concourse.bacc as bacc
import concourse.bass as bass
import concourse.mybir as mybir
import concourse.tile as tile
from concourse.masks import make_identity

# Problem shape (hardcoded per the task contract).
N = 2048          # batch
J = 8             # context window (2*CTX)
D = 256           # hidden
V = 50000         # vocab
C = 8             # cores
VS = V // C       # vocab shard per core = 6250

P = 128
NB = N // P       # 16 row-blocks
LB = NB // C      # local blocks per core = 2
VT = 1024         # eviction tile width (two PSUM banks)

F32 = mybir.dt.float32
BF16 = mybir.dt.bfloat16
I32 = mybir.dt.int32

_CACHE = {}


def _build(zero_bias=True):
    """Build + compile the single-core SPMD Bass program."""
    key = ("nc", zero_bias)
    if key in _CACHE:
        return _CACHE[key]

    nc = bacc.Bacc("TRN2", target_bir_lowering=False, debug=False, num_devices=C)

    idx_d = nc.dram_tensor("idx", [P, LB * J], I32, kind="ExternalInput")
    w1t_d = nc.dram_tensor("w1t", [V + 1, D], BF16, kind="ExternalInput")
    w2t_d = nc.dram_tensor("w2t", [D, VS], BF16, kind="ExternalInput")
    out_d = nc.dram_tensor("out", [N, VS], BF16, kind="ExternalOutput")
    # per-local-block hT staging + the all-gathered copy (CC needs HBM->HBM)
    hpart = [nc.dram_tensor(f"hpart{lb}", [2, P, P], BF16) for lb in range(LB)]
    hall = [nc.dram_tensor(f"hall{lb}", [C, 2, P, P], BF16,
                           kind="Internal", addr_space="Shared")
            for lb in range(LB)]
    if not zero_bias:
        b1_d = nc.dram_tensor("b1", [2, P, 1], F32, kind="ExternalInput")
        b2_d = nc.dram_tensor("b2", [1, VS], F32, kind="ExternalInput")

    # 6 x 1024 + 106; evictions alternate DVE / Scalar per tile
    vsub = [(v, min(VT, VS - v)) for v in range(0, VS, VT)]

    with tile.TileContext(nc) as tc, ExitStack() as ctx:
        const = ctx.enter_context(tc.tile_pool(name="const", bufs=1))
        gpool = ctx.enter_context(tc.tile_pool(name="g8", bufs=2))
        t4pool = ctx.enter_context(tc.tile_pool(name="t4", bufs=2))
        t2pool = ctx.enter_context(tc.tile_pool(name="t2", bufs=2))
        hpool = ctx.enter_context(tc.tile_pool(name="hraw", bufs=2))
        htpool = ctx.enter_context(tc.tile_pool(name="ht", bufs=4))
        hgpool = ctx.enter_context(tc.tile_pool(name="hg", bufs=6))
        opool = ctx.enter_context(tc.tile_pool(name="out", bufs=2))
        ps_s = ctx.enter_context(tc.tile_pool(name="ps_s", bufs=2, space="PSUM"))
        ps_b = ctx.enter_context(tc.tile_pool(name="ps_b", bufs=3, space="PSUM"))

        # ---- resident tensors -------------------------------------------
        idx_sb = const.tile([P, LB * J], I32, tag="idx")
        nc.sync.dma_start(idx_sb[:], idx_d[:])
        ident = const.tile([P, P], BF16, tag="ident")
        make_identity(nc, ident[:])
        w2sb = [const.tile([P, VS], BF16, tag=f"w2{h}", name=f"w2{h}")
                for h in (0, 1)]
        for h in (0, 1):
            nc.sync.dma_start(w2sb[h][:], w2t_d[h * P:(h + 1) * P, :])
        if not zero_bias:
            b1t = [const.tile([P, 1], F32, tag=f"b1{h}", name=f"b1{h}")
                   for h in (0, 1)]
            for h in (0, 1):
                nc.sync.dma_start(b1t[h][:], b1_d[h])
            b2bc = const.tile([P, VS], F32, tag="b2bc")
            nc.sync.dma_start(b2bc[:], b2_d[:].to_broadcast([P, VS]))

        # ---- layer 1: own blocks, then stagger the collectives ----------
        def layer1(lb):
            # 8 independent one-row-per-partition HW-DGE gathers
            g8 = gpool.tile([P, J, D], BF16, tag="g8", name="g8")
            for j in range(J):
                nc.gpsimd.indirect_dma_start(
                    out=g8[:, j, :],
                    out_offset=None,
                    in_=w1t_d[:],
                    in_offset=bass.IndirectOffsetOnAxis(
                        ap=idx_sb[:, lb * J + j:lb * J + j + 1], axis=0),
                )
            t4 = t4pool.tile([P, 4, D], BF16, tag="t4", name="t4")
            nc.vector.tensor_add(t4[:], g8[:, 0:4, :], g8[:, 4:8, :])
            t2 = t2pool.tile([P, 2, D], BF16, tag="t2", name="t2")
            nc.vector.tensor_add(t2[:], t4[:, 0:2, :], t4[:, 2:4, :])
            h_raw = hpool.tile([P, D], BF16, tag="hraw", name="h_raw")
            nc.vector.tensor_add(h_raw[:], t2[:, 0, :], t2[:, 1, :])

            for h in (0, 1):
                pt = ps_s.tile([P, P], BF16, tag="ps", name="pt")
                nc.tensor.transpose(pt[:], h_raw[:, h * P:(h + 1) * P],
                                    ident[:])
                ht = htpool.tile([P, P], BF16, tag="ht", name="ht")
                if zero_bias:
                    nc.vector.tensor_scalar_max(ht[:], pt[:], 0.0)
                else:
                    nc.scalar.activation(ht[:], pt[:],
                                         mybir.ActivationFunctionType.Relu,
                                         bias=b1t[h][:], scale=1.0)
                nc.sync.dma_start(hpart[lb][h], ht[:])
            nc.gpsimd.collective_compute(
                "AllGather",
                mybir.AluOpType.bypass,
                replica_groups=[list(range(C))],
                ins=[hpart[lb][:]],
                outs=[hall[lb][:]],
            )

        for lb in range(LB):
            layer1(lb)

        # ---- layer 2: all 16 blocks from the gathered hT ----------------
        def readback(nb):
            src, lb = nb // LB, nb % LB
            hts = []
            for h in (0, 1):
                hg = hgpool.tile([P, P], BF16, tag="hg", name="hg")
                nc.sync.dma_start(hg[:], hall[lb][src, h])
                hts.append(hg)
            return hts

        def layer2(nb, ht):
            ob = opool.tile([P, VS], BF16, tag="ob", name="ob")
            for t, (v0, vw) in enumerate(vsub):
                po = ps_b.tile([P, VT], F32, tag="po", name="po")
                for sub in range(0, vw, 512):
                    sw = min(512, vw - sub)
                    for h in (0, 1):
                        nc.tensor.matmul(
                            po[:, sub:sub + sw],
                            lhsT=ht[h][:],
                            rhs=w2sb[h][:, v0 + sub:v0 + sub + sw],
                            start=(h == 0),
                            stop=(h == 1))
                if not zero_bias:
                    nc.vector.tensor_add(ob[:, v0:v0 + vw], po[:, :vw],
                                         b2bc[:, v0:v0 + vw])
                elif t % 2 == 0 and t < 6:
                    nc.vector.tensor_scalar_add(ob[:, v0:v0 + vw],
                                                po[:, :vw], 0.0)
                else:
                    nc.scalar.copy(ob[:, v0:v0 + vw], po[:, :vw])
            nc.sync.dma_start(out_d[nb * P:(nb + 1) * P, :], ob[:])

        PREF = 2
        hts = {nb: readback(nb) for nb in range(PREF)}
        for nb in range(NB):
            if nb + PREF < NB:
                hts[nb + PREF] = readback(nb + PREF)
            layer2(nb, hts.pop(nb))

    nc.compile()
    _CACHE[key] = nc
    return nc


def _host_prep(inputs, W1, b1, W2, b2, zero_bias):
    x = np.asarray(inputs)
    assert x.shape == (N, J) and x.dtype == np.int32

    # duplicate mask: scatter-SET semantics -> only first occurrence counts;
    # duplicates are redirected to the all-zero row V of the augmented W1T.
    dup = np.zeros((N, J), dtype=bool)
    for j in range(1, J):
        dup[:, j] = (x[:, :j] == x[:, j:j + 1]).any(axis=1)
    xd = np.where(dup, V, x).astype(np.int32)

    w1 = np.asarray(W1, dtype=np.float32)
    w1t = np.concatenate([w1.T / J, np.zeros((1, D), np.float32)], axis=0)
    w1t = np.ascontiguousarray(w1t).astype(ml_dtypes.bfloat16)   # [V+1, D]

    w2t = np.ascontiguousarray(
        np.asarray(W2, dtype=np.float32).T).astype(ml_dtypes.bfloat16)

    in_maps = []
    for c in range(C):
        sl = slice(c * VS, (c + 1) * VS)
        # idx2[p, lb*J + j] = xd[c*256 + lb*128 + p, j]
        xc = xd[c * LB * P:(c + 1) * LB * P]
        idx2 = np.ascontiguousarray(
            xc.reshape(LB, P, J).transpose(1, 0, 2).reshape(P, LB * J))
        m = {
            "idx": idx2,
            "w1t": w1t,
            "w2t": np.ascontiguousarray(w2t[:, sl]),
        }
        if not zero_bias:
            m["b1"] = np.ascontiguousarray(
                np.asarray(b1, dtype=np.float32).reshape(2, P, 1))
            m["b2"] = np.ascontiguousarray(
                np.asarray(b2, dtype=np.float32)[sl].reshape(1, VS))
        in_maps.append(m)
    return in_maps


def run(inputs, W1, b1, W2, b2, trace=False):
    from concourse.bass_utils import run_bass_kernel_spmd

    zero_bias = not (np.any(np.asarray(b1)) or np.any(np.asarray(b2)))
    nc = _build(zero_bias)
    in_maps = _host_prep(inputs, W1, b1, W2, b2, zero_bias)
    res = run_bass_kernel_spmd(nc, in_maps, core_ids=list(range(C)), trace=trace)
    out = np.concatenate([res.results[c]["out"] for c in range(C)], axis=1)
    return out.astype(np.float32), res


def kernel(inputs, W1, b1, W2, b2):
    out, _ = run(inputs, W1, b1, W2, b2, trace=False)
    return out


# revision 5
# speedup vs baseline: 2.3791x; 1.2193x over previous
"""CBOW forward (embedding lookup -> ReLU -> vocab projection) on 8 TRN2 cores.

Full inputs in, full output out.  Sharding: pure data-parallel over the
batch.  Core c owns rows [c*256, (c+1)*256): it gathers + reduces the
context embeddings for its two 128-row blocks (16 HW-DGE indirect-DMA
calls -- the indirect path costs ~1.4us of gpsimd issue time per call
regardless of size, so call count is what matters), relu's the
transposed result into four resident bf16 hT tiles, then computes
out[own, :] = h @ W2.T for the FULL vocab, streaming W2T through SBUF
in [128, 1024] bf16 tiles.

Why not vocab-shard layer 2 (8x less W2 traffic)?  That needs an
AllGather of h, and a measured probe puts the fixed cost of any
collective in this runtime at ~95us (rendezvous + init) -- more than
the W2 streaming it saves.  With no cross-core dependency, per-core
launch skew doesn't stack either.

The harness accuracy gate is rel_err < 2e-2; the whole pipeline runs in
bf16 (measured ~5e-3): W1T is pre-scaled by 1/8 (exact in bf16), the
context sum is a 3-level bf16 DVE tree, layer 2 is a single bf16 term
with fp32 PSUM accumulate, and the output is written bf16 (halves the
dominant HBM traffic; host upcasts).  PSUM eviction (fp32->bf16,
~1.1ns/col/partition) alternates DVE/Scalar per tile so neither engine
paces the loop.  Output accumulates in [128, 8192] chunks so every DMA
descriptor row is 16KB contiguous.

Duplicate context indices use scatter-SET semantics (count once): the
host redirects duplicate occurrences to an appended all-zero row of
W1T.  b1/b2 are zero in this problem (spec fill=zeros); a general
fallback path (scalar-relu with b1 bias, streamed b2-add evictions)
compiles only if nonzero biases ever show up.
"""

from contextlib import ExitStack

import numpy as np
import ml_dtypes

import concourse.bacc as bacc
import concourse.bass as bass
import concourse.mybir as mybir
import concourse.tile as tile
from concourse.masks import make_identity

# Problem shape (hardcoded per the task contract).
N = 2048          # batch
J = 8             # context window (2*CTX)
D = 256           # hidden
V = 50000         # vocab
C = 8             # cores

P = 128
LB = N // (C * P)  # local 128-row blocks per core = 2
VT = 1024          # matmul/eviction tile width (two PSUM banks)
CHW = 8192         # output chunk width (16KB bf16 rows -> fat DMA descriptors)

F32 = mybir.dt.float32
BF16 = mybir.dt.bfloat16
I32 = mybir.dt.int32

_CACHE = {}


def _build(zero_bias=True):
    """Build + compile the single-core SPMD Bass program."""
    key = ("nc", zero_bias)
    if key in _CACHE:
        return _CACHE[key]

    nc = bacc.Bacc("TRN2", target_bir_lowering=False, debug=False, num_devices=C)

    idx_d = nc.dram_tensor("idx", [P, LB * J], I32, kind="ExternalInput")
    w1t_d = nc.dram_tensor("w1t", [V + 1, D], BF16, kind="ExternalInput")
    w2t_d = nc.dram_tensor("w2t", [D, V], BF16, kind="ExternalInput")
    out_d = nc.dram_tensor("out", [LB * P, V], BF16, kind="ExternalOutput")
    if not zero_bias:
        b1_d = nc.dram_tensor("b1", [2, P, 1], F32, kind="ExternalInput")
        b2_d = nc.dram_tensor("b2", [1, V], F32, kind="ExternalInput")

    # output chunks of 8192 cols; vtiles of 1024 within a chunk (tail 848)
    chunks = [(k, min(CHW, V - k)) for k in range(0, V, CHW)]

    with tile.TileContext(nc) as tc, ExitStack() as ctx:
        const = ctx.enter_context(tc.tile_pool(name="const", bufs=1))
        w2pool = ctx.enter_context(tc.tile_pool(name="w2", bufs=10))
        gpool = ctx.enter_context(tc.tile_pool(name="g8", bufs=2))
        t4pool = ctx.enter_context(tc.tile_pool(name="t4", bufs=2))
        t2pool = ctx.enter_context(tc.tile_pool(name="t2", bufs=2))
        hpool = ctx.enter_context(tc.tile_pool(name="hraw", bufs=2))
        opool = ctx.enter_context(tc.tile_pool(name="out", bufs=4))
        b2pool = ctx.enter_context(tc.tile_pool(name="b2s", bufs=6))
        ps_s = ctx.enter_context(tc.tile_pool(name="ps_s", bufs=2, space="PSUM"))
        ps_b = ctx.enter_context(tc.tile_pool(name="ps_b", bufs=3, space="PSUM"))

        # ---- resident tensors -------------------------------------------
        idx_sb = const.tile([P, LB * J], I32, tag="idx")
        nc.sync.dma_start(idx_sb[:], idx_d[:])
        ident = const.tile([P, P], BF16, tag="ident")
        make_identity(nc, ident[:])
        if not zero_bias:
            b1t = [const.tile([P, 1], F32, tag=f"b1{h}", name=f"b1{h}")
                   for h in (0, 1)]
            for h in (0, 1):
                nc.sync.dma_start(b1t[h][:], b1_d[h])

        # ---- layer 1: own two blocks ------------------------------------
        def layer1(lb):
            g8 = gpool.tile([P, J, D], BF16, tag="g8", name="g8")
            for j in range(J):
                nc.gpsimd.indirect_dma_start(
                    out=g8[:, j, :],
                    out_offset=None,
                    in_=w1t_d[:],
                    in_offset=bass.IndirectOffsetOnAxis(
                        ap=idx_sb[:, lb * J + j:lb * J + j + 1], axis=0),
                )
            t4 = t4pool.tile([P, 4, D], BF16, tag="t4", name="t4")
            nc.vector.tensor_add(t4[:], g8[:, 0:4, :], g8[:, 4:8, :])
            t2 = t2pool.tile([P, 2, D], BF16, tag="t2", name="t2")
            nc.vector.tensor_add(t2[:], t4[:, 0:2, :], t4[:, 2:4, :])
            h_raw = hpool.tile([P, D], BF16, tag="hraw", name="h_raw")
            nc.vector.tensor_add(h_raw[:], t2[:, 0, :], t2[:, 1, :])

            hts = []
            for h in (0, 1):
                pt = ps_s.tile([P, P], BF16, tag="ps", name="pt")
                nc.tensor.transpose(pt[:], h_raw[:, h * P:(h + 1) * P],
                                    ident[:])
                ht = const.tile([P, P], BF16, tag=f"ht{lb}{h}",
                                name=f"ht{lb}{h}")
                if zero_bias:
                    nc.vector.tensor_scalar_max(ht[:], pt[:], 0.0)
                else:
                    nc.scalar.activation(ht[:], pt[:],
                                         mybir.ActivationFunctionType.Relu,
                                         bias=b1t[h][:], scale=1.0)
                hts.append(ht)
            return hts

        ht = [layer1(lb) for lb in range(LB)]

        # ---- layer 2: stream W2T, both blocks per vtile ------------------
        def fetch_w2(v0, vw):
            pair = []
            for h in (0, 1):
                w2 = w2pool.tile([P, VT], BF16, tag="w2", name="w2")
                nc.sync.dma_start(w2[:, :vw], w2t_d[h * P:(h + 1) * P,
                                                    v0:v0 + vw])
                pair.append(w2)
            if not zero_bias:
                b2s = b2pool.tile([P, VT], F32, tag="b2s", name="b2s")
                nc.sync.dma_start(b2s[:, :vw],
                                  b2_d[:, v0:v0 + vw].to_broadcast([P, vw]))
                pair.append(b2s)
            return pair

        vtiles = []
        for k0, kw in chunks:
            for v0 in range(k0, k0 + kw, VT):
                vtiles.append((v0, min(VT, k0 + kw - v0)))

        PREF = 3
        w2f = {i: fetch_w2(*vtiles[i]) for i in range(PREF)}
        ob = {}
        vt_i = 0
        for k, (k0, kw) in enumerate(chunks):
            for lb in range(LB):
                ob[lb] = opool.tile([P, CHW], BF16, tag=f"ob{lb}",
                                    name=f"ob{lb}")
            nvt = (kw + VT - 1) // VT
            for t in range(nvt):
                v0, vw = vtiles[vt_i]
                if vt_i + PREF < len(vtiles):
                    w2f[vt_i + PREF] = fetch_w2(*vtiles[vt_i + PREF])
                w2pair = w2f.pop(vt_i)
                c0 = v0 - k0
                for lb in range(LB):
                    po = ps_b.tile([P, VT], F32, tag="po", name="po")
                    for sub in range(0, vw, 512):
                        sw = min(512, vw - sub)
                        for h in (0, 1):
                            nc.tensor.matmul(
                                po[:, sub:sub + sw],
                                lhsT=ht[lb][h][:],
                                rhs=w2pair[h][:, sub:sub + sw],
                                start=(h == 0),
                                stop=(h == 1))
                    if not zero_bias:
                        nc.vector.tensor_add(ob[lb][:, c0:c0 + vw],
                                             po[:, :vw], w2pair[2][:, :vw])
                    elif (lb + t) % 2 == 0:
                        nc.vector.tensor_scalar_add(ob[lb][:, c0:c0 + vw],
                                                    po[:, :vw], 0.0)
                    else:
                        nc.scalar.copy(ob[lb][:, c0:c0 + vw], po[:, :vw])
                vt_i += 1
            for lb in range(LB):
                nc.sync.dma_start(out_d[lb * P:(lb + 1) * P, k0:k0 + kw],
                                  ob[lb][:, :kw])

    nc.compile()
    _CACHE[key] = nc
    return nc


def _host_prep(inputs, W1, b1, W2, b2, zero_bias):
    x = np.asarray(inputs)
    assert x.shape == (N, J) and x.dtype == np.int32

    # duplicate mask: scatter-SET semantics -> only first occurrence counts;
    # duplicates are redirected to the all-zero row V of the augmented W1T.
    dup = np.zeros((N, J), dtype=bool)
    for j in range(1, J):
        dup[:, j] = (x[:, :j] == x[:, j:j + 1]).any(axis=1)
    xd = np.where(dup, V, x).astype(np.int32)

    w1 = np.asarray(W1, dtype=np.float32)
    w1t = np.concatenate([w1.T / J, np.zeros((1, D), np.float32)], axis=0)
    w1t = np.ascontiguousarray(w1t).astype(ml_dtypes.bfloat16)   # [V+1, D]

    w2t = np.ascontiguousarray(
        np.asarray(W2, dtype=np.float32).T).astype(ml_dtypes.bfloat16)

    in_maps = []
    for c in range(C):
        # idx2[p, lb*J + j] = xd[c*256 + lb*128 + p, j]
        xc = xd[c * LB * P:(c + 1) * LB * P]
        idx2 = np.ascontiguousarray(
            xc.reshape(LB, P, J).transpose(1, 0, 2).reshape(P, LB * J))
        m = {
            "idx": idx2,
            "w1t": w1t,
            "w2t": w2t,
        }
        if not zero_bias:
            m["b1"] = np.ascontiguousarray(
                np.asarray(b1, dtype=np.float32).reshape(2, P, 1))
            m["b2"] = np.ascontiguousarray(
                np.asarray(b2, dtype=np.float32).reshape(1, V))
        in_maps.append(m)
    return in_maps


def run(inputs, W1, b1, W2, b2, trace=False):
    from concourse.bass_utils import run_bass_kernel_spmd

    zero_bias = not (np.any(np.asarray(b1)) or np.any(np.asarray(b2)))
    nc = _build(zero_bias)
    in_maps = _host_prep(inputs, W1, b1, W2, b2, zero_bias)
    res = run_bass_kernel_spmd(nc, in_maps, core_ids=list(range(C)), trace=trace)
    out = np.concatenate([res.results[c]["out"] for c in range(C)], axis=0)
    return out.astype(np.float32), res


def kernel(inputs, W1, b1, W2, b2):
    out, _ = run(inputs, W1, b1, W2, b2, trace=False)
    return out
